# revision 2
# baseline (speedup 1.0000x reference)
"""Trainium2 Bass kernel for nn_DGM_c (DGM graph-construction layer).

Reference computation (see problem statement):
    x_emb = relu(A @ x @ W + b)                       [B,N,E]
    centroid = mean(x_emb, axis=-2); scale = 0.9/max|x_emb-centroid|
    xs = (x_emb-centroid)*scale
    D = cdist(xs)^2 ; adj = sigmoid(T*(|thr| - D))
    edge_index = fixed arange pattern ; edges_weight = adj.reshape(-1)

Key identity: the centroid cancels in pairwise differences, so
    D_ij = scale^2 * (|e_i|^2 + |e_j|^2 - 2 e_i.e_j),  e = x_emb.
Only one global scalar (scale) couples all rows, so we run two SPMD
launches over 8 NeuronCores (core c -> batch c//4, row-block c%4 of 512
rows) with a tiny host step between them:

  Launch 1 (per core): t^T = x_b^T A_blk^T via 16 accumulated fp32
    matmuls (A pre-transposed on host so the contraction dim lands on
    partitions), then x_emb_blk^T = relu(W^T t^T + b) -> [64, 512].
    x_b and A_blk^T are packed into one DRAM tensor, loaded in 2 big
    DMAs (DMA issue overhead is the dominant modeled cost).
  Host: assemble x_emb, compute scale/sq norms, fold every constant into
    small augmented operands.
  Launch 2 (per core): one K=65 matmul per [128,512] tile computes
    2*T*scale^2*G - T*scale^2*sq_j; ScalarE applies
    sigmoid(psum + (T|thr| - T*scale^2*sq_i)); results are staged in
    SBUF and written out in 2 x 2MiB DMAs.

edge_index is input-independent -> generated host-side.
"""

import os
import sys
from contextlib import ExitStack

for _p in ("/opt/trn_rl_repo", "/root/.axon_site/_ro/trn_rl_repo"):
    if os.path.isdir(_p) and _p not in sys.path:
        sys.path.insert(0, _p)

import numpy as np

import concourse.bass as bass  # noqa: F401  (registers engines)
import concourse.tile as tile
from concourse import bacc, mybir
from concourse.bass_utils import run_bass_kernel_spmd

B, N, F_IN, F_EMB = 2, 2048, 128, 64
NCORES = 8
CPB = NCORES // B          # cores per batch
R = N // CPB               # rows per core = 512
KT = N // 128              # contraction tiles = 16
IT = R // 128              # row tiles per core = 4
JT = N // 512              # column chunks per core = 4
C1 = F_IN + R              # packed row width of phase-1 input = 640
HK = KT // 2               # k-tiles per load chunk = 8
F32 = mybir.dt.float32
CORE_IDS = list(range(NCORES))

_NC_CACHE: dict = {}


def _build_phase1():
    """x_emb_blk^T = relu(W^T (x_b^T A_blk^T) + b) for this core's rows."""
    nc = bacc.Bacc("TRN2", target_bir_lowering=False, debug=False,
                   num_devices=NCORES)
    # pk[n, 0:F_IN] = x_b[n, :]; pk[n, F_IN:] = A[b, rows, n]  (A_blk^T)
    pk_ap = nc.dram_tensor("pk", [N, C1], F32, kind="ExternalInput").ap()
    # wb[:, 0:F_EMB] = W_embed; wb[0:F_EMB, F_EMB] = b_embed
    wb_ap = nc.dram_tensor("wb", [F_IN, F_EMB + 1], F32,
                           kind="ExternalInput").ap()
    et_ap = nc.dram_tensor("et", [F_EMB, R], F32, kind="ExternalOutput").ap()

    with tile.TileContext(nc) as tc, ExitStack() as ctx:
        const = ctx.enter_context(tc.tile_pool(name="const", bufs=1))
        pkp = ctx.enter_context(tc.tile_pool(name="pkp", bufs=2))
        pst = ctx.enter_context(tc.tile_pool(name="pst", bufs=1, space="PSUM"))
        pse = ctx.enter_context(tc.tile_pool(name="pse", bufs=1, space="PSUM"))
        spool = ctx.enter_context(tc.tile_pool(name="sp", bufs=2))

        wb = const.tile([F_IN, F_EMB + 1], F32)
        nc.sync.dma_start(wb[:], wb_ap[:])

        pk_r = pk_ap.rearrange("(k p) c -> p k c", p=128)   # [128, KT, C1]
        psum_t = pst.tile([128, R], F32)
        chunks = []
        for h in range(2):
            ch = pkp.tile([128, HK * C1], F32, tag="chunk")
            nc.sync.dma_start(
                ch[:].rearrange("p (k c) -> p k c", k=HK),
                pk_r[:, h * HK:(h + 1) * HK, :],
            )
            chunks.append(ch)
        for k in range(KT):
            ch, kk = chunks[k // HK], k % HK
            nc.tensor.matmul(
                psum_t[:],
                ch[:, kk * C1:kk * C1 + F_IN],            # x tile  [128,128]
                ch[:, kk * C1 + F_IN:(kk + 1) * C1],      # A^T tile [128,512]
                start=(k == 0), stop=(k == KT - 1),
            )
        tts = spool.tile([128, R], F32)
        nc.vector.tensor_copy(tts[:], psum_t[:])
        psum_e = pse.tile([F_EMB, R], F32)
        nc.tensor.matmul(psum_e[:], wb[:, 0:F_EMB], tts[:],
                         start=True, stop=True)
        esb = spool.tile([F_EMB, R], F32)
        nc.scalar.activation(esb[:], psum_e[:],
                             mybir.ActivationFunctionType.Relu,
                             bias=wb[0:F_EMB, F_EMB:F_EMB + 1])
        nc.sync.dma_start(et_ap[:], esb[:])

    nc.compile()
    return nc


def _build_phase2():
    """w_blk = sigmoid(lhsT_aug^T @ rhs_aug + bias_i) for this core's rows."""
    nc = bacc.Bacc("TRN2", target_bir_lowering=False, debug=False,
                   num_devices=NCORES)
    K = F_EMB + 1
    # lr[:, 0:R] = lhsT_aug ; lr[:, R:] = rhs_aug
    lr_ap = nc.dram_tensor("lr", [K, R + N], F32, kind="ExternalInput").ap()
    bi_ap = nc.dram_tensor("bi", [128, IT], F32, kind="ExternalInput").ap()
    wo_ap = nc.dram_tensor("wo", [R, N], F32, kind="ExternalOutput").ap()

    with tile.TileContext(nc) as tc, ExitStack() as ctx:
        inp = ctx.enter_context(tc.tile_pool(name="inp", bufs=1))
        psp = ctx.enter_context(tc.tile_pool(name="psp", bufs=4, space="PSUM"))
        outp = ctx.enter_context(tc.tile_pool(name="outp", bufs=2))

        lr = inp.tile([K, R + N], F32)
        nc.sync.dma_start(lr[:], lr_ap[:])
        bi = inp.tile([128, IT], F32)
        nc.sync.dma_start(bi[:], bi_ap[:])

        wo_r = wo_ap.rearrange("(i p) n -> p i n", p=128)   # [128, IT, N]
        for h in range(2):                                  # halves: 2 i-tiles
            wsb = outp.tile([128, 2 * N], F32, tag="wsb")
            for ii in range(2):
                i = 2 * h + ii
                for j in range(JT):
                    ps = psp.tile([128, 512], F32, tag="ps")
                    nc.tensor.matmul(
                        ps[:], lr[:, i * 128:(i + 1) * 128],
                        lr[:, R + j * 512:R + (j + 1) * 512],
                        start=True, stop=True,
                    )
                    nc.scalar.activation(
                        wsb[:, ii * N + j * 512:ii * N + (j + 1) * 512],
                        ps[:], mybir.ActivationFunctionType.Sigmoid,
                        bias=bi[:, i:i + 1])
            nc.sync.dma_start(
                wo_r[:, 2 * h:2 * h + 2, :],
                wsb[:].rearrange("p (i n) -> p i n", i=2),
            )

    nc.compile()
    return nc


def _get_nc(key, builder):
    nc = _NC_CACHE.get(key)
    if nc is None:
        nc = builder()
        _NC_CACHE[key] = nc
    return nc


def _edge_index() -> np.ndarray:
    idx = np.arange(B * N * N, dtype=np.int32)
    rows = idx // N
    cols = idx % N + N * (rows // N)
    return np.stack([rows, cols]).astype(np.int32)


def kernel(x, A, W_embed, b_embed, temperature, threshold):
    x = np.asarray(x, dtype=np.float32)
    A = np.asarray(A, dtype=np.float32)
    W_embed = np.asarray(W_embed, dtype=np.float32)
    b_embed = np.asarray(b_embed, dtype=np.float32)
    T = np.float32(np.asarray(temperature).reshape(()))
    thr = np.abs(np.float32(np.asarray(threshold).reshape(())))

    # ---- launch 1: x_emb ----
    nc1 = _get_nc("p1", _build_phase1)
    wb = np.empty((F_IN, F_EMB + 1), dtype=np.float32)
    wb[:, :F_EMB] = W_embed
    wb[:, F_EMB] = 0.0
    wb[:F_EMB, F_EMB] = b_embed
    in1 = []
    for c in range(NCORES):
        b, rb = divmod(c, CPB)
        pk = np.empty((N, C1), dtype=np.float32)
        pk[:, :F_IN] = x[b]
        pk[:, F_IN:] = A[b, rb * R:(rb + 1) * R, :].T
        in1.append({"pk": pk, "wb": wb})
    res1 = run_bass_kernel_spmd(nc1, in1, core_ids=CORE_IDS)

    x_emb = np.empty((B, N, F_EMB), dtype=np.float32)
    for c in range(NCORES):
        b, rb = divmod(c, CPB)
        x_emb[b, rb * R:(rb + 1) * R, :] = res1.results[c]["et"].T

    # ---- host: global scale + fold constants ----
    centroid = x_emb.mean(axis=1, keepdims=True, dtype=np.float32)
    scale = np.float32(0.9) / np.abs(x_emb - centroid).max()
    s2 = np.float32(T * scale * scale)          # T * scale^2
    sq0 = np.einsum("bne,bne->bn", x_emb, x_emb).astype(np.float32)  # [B,N]

    nc2 = _get_nc("p2", _build_phase2)
    in2 = []
    for c in range(NCORES):
        b, rb = divmod(c, CPB)
        eT = x_emb[b].T                          # [E, N]
        lr = np.empty((F_EMB + 1, R + N), dtype=np.float32)
        lr[:F_EMB, :R] = (2.0 * s2) * eT[:, rb * R:(rb + 1) * R]
        lr[F_EMB, :R] = 1.0
        lr[:F_EMB, R:] = eT
        lr[F_EMB, R:] = (-s2) * sq0[b]
        bi = (T * thr - s2 * sq0[b, rb * R:(rb + 1) * R])
        bi = np.ascontiguousarray(bi.reshape(IT, 128).T)   # [128, IT]
        in2.append({"lr": lr, "bi": bi})
    res2 = run_bass_kernel_spmd(nc2, in2, core_ids=CORE_IDS)

    adj = np.empty((B, N, N), dtype=np.float32)
    for c in range(NCORES):
        b, rb = divmod(c, CPB)
        adj[b, rb * R:(rb + 1) * R, :] = res2.results[c]["wo"]

    return x_emb, _edge_index(), adj.reshape(-1)


# revision 7
# speedup vs baseline: 1.6868x; 1.6868x over previous
"""Trainium2 Bass kernel for nn_DGM_c (DGM graph-construction layer).

Reference computation (see problem statement):
    x_emb = relu(A @ x @ W + b)                       [B,N,E]
    centroid = mean(x_emb, axis=-2); scale = 0.9/max|x_emb-centroid|
    xs = (x_emb-centroid)*scale
    D = cdist(xs)^2 ; adj = sigmoid(T*(|thr| - D))
    edge_index = fixed arange pattern ; edges_weight = adj.reshape(-1)

Key identity: the centroid cancels in pairwise differences, so
    D_ij = scale^2 * (|e_i|^2 + |e_j|^2 - 2 e_i.e_j),  e = x_emb.
Only one global scalar (scale) couples all rows, so we run two SPMD
launches over 8 NeuronCores (core c -> batch c//4, row-block c%4 of 512
rows) with a tiny host step between them:

  Launch 1 (per core): t^T = x_b^T A_blk^T via 16 accumulated fp32
    matmuls (A pre-transposed on host so the contraction dim lands on
    partitions), then x_emb_blk^T = relu(W^T t^T + b) -> [64, 512].
    x_b and A_blk^T are packed into one DRAM tensor, loaded in 2 big
    DMAs (DMA issue overhead is the dominant modeled cost).
  Host: assemble x_emb, compute scale/sq norms, fold every constant into
    small augmented operands.
  Launch 2 (per core): one K=65 matmul per [128,512] tile computes
    2*T*scale^2*G - T*scale^2*sq_j; ScalarE applies
    sigmoid(psum + (T|thr| - T*scale^2*sq_i)); results are staged in
    SBUF and written out in 2 x 2MiB DMAs.

edge_index is input-independent -> generated host-side.
"""

import os
import sys
from contextlib import ExitStack

for _p in ("/opt/trn_rl_repo", "/root/.axon_site/_ro/trn_rl_repo"):
    if os.path.isdir(_p) and _p not in sys.path:
        sys.path.insert(0, _p)

import numpy as np

import concourse.bass as bass  # noqa: F401  (registers engines)
import concourse.tile as tile
from concourse import bacc, mybir
from concourse.bass_utils import run_bass_kernel_spmd

B, N, F_IN, F_EMB = 2, 2048, 128, 64
NCORES = 8
CPB = NCORES // B          # cores per batch
R = N // CPB               # rows per core = 512
KT = N // 128              # contraction tiles = 16
IT = R // 128              # row tiles per core = 4
JT = N // 512              # column chunks per core = 4
C1 = F_IN + R              # packed row width of phase-1 input = 640
NCHUNK = 4                 # phase-1 load chunks
HK = KT // NCHUNK          # k-tiles per load chunk = 4
F32 = mybir.dt.float32
F32R = mybir.dt.float32r   # fp32 bits, fast PE mode (1 cycle/row at N>=256)
CORE_IDS = list(range(NCORES))

_NC_CACHE: dict = {}


def _build_phase1():
    """x_emb_blk^T = relu(W^T (x_b^T A_blk^T) + b) for this core's rows."""
    nc = bacc.Bacc("TRN2", target_bir_lowering=False, debug=False,
                   num_devices=NCORES)
    # pk[n, 0:F_IN] = x_b[n, :]; pk[n, F_IN:] = A[b, rows, n]  (A_blk^T)
    pk_ap = nc.dram_tensor("pk", [N, C1], F32R, kind="ExternalInput").ap()
    # wb[:, 0:F_EMB] = W_embed; wb[0:F_EMB, F_EMB] = b_embed
    wb_ap = nc.dram_tensor("wb", [F_IN, F_EMB + 1], F32R,
                           kind="ExternalInput").ap()
    et_ap = nc.dram_tensor("et", [F_EMB, R], F32, kind="ExternalOutput").ap()

    with tile.TileContext(nc) as tc, ExitStack() as ctx:
        const = ctx.enter_context(tc.tile_pool(name="const", bufs=1))
        pkp = ctx.enter_context(tc.tile_pool(name="pkp", bufs=2))
        pst = ctx.enter_context(tc.tile_pool(name="pst", bufs=1, space="PSUM"))
        pse = ctx.enter_context(tc.tile_pool(name="pse", bufs=1, space="PSUM"))
        spool = ctx.enter_context(tc.tile_pool(name="sp", bufs=2))

        wb = const.tile([F_IN, F_EMB + 1], F32R)
        nc.sync.dma_start(wb[:], wb_ap[:])

        pk_r = pk_ap.rearrange("(k p) c -> p k c", p=128)   # [128, KT, C1]
        psum_t = pst.tile([128, R], F32)
        chunks = []
        for h in range(NCHUNK):
            ch = pkp.tile([128, HK * C1], F32R, tag="chunk")
            nc.sync.dma_start(
                ch[:].rearrange("p (k c) -> p k c", k=HK),
                pk_r[:, h * HK:(h + 1) * HK, :],
            )
            chunks.append(ch)
        for k in range(KT):
            ch, kk = chunks[k // HK], k % HK
            nc.tensor.matmul(
                psum_t[:],
                ch[:, kk * C1:kk * C1 + F_IN],                      # x tile
                ch[:, kk * C1 + F_IN:(kk + 1) * C1],                # A^T tile
                start=(k == 0), stop=(k == KT - 1),
            )
        tts = spool.tile([128, R], F32R)
        nc.vector.tensor_copy(tts[:], psum_t[:])
        psum_e = pse.tile([F_EMB, R], F32)
        nc.tensor.matmul(psum_e[:], wb[:, 0:F_EMB], tts[:],
                         start=True, stop=True)
        esb = spool.tile([F_EMB, R], F32)
        nc.scalar.activation(esb[:], psum_e[:],
                             mybir.ActivationFunctionType.Relu,
                             bias=wb[0:F_EMB, F_EMB:F_EMB + 1].bitcast(F32))
        nc.sync.dma_start(et_ap[:], esb[:])

    nc.compile()
    return nc


def _build_phase2():
    """w_blk = sigmoid(lhsT_aug^T @ rhs_aug + bias_i) for this core's rows."""
    nc = bacc.Bacc("TRN2", target_bir_lowering=False, debug=False,
                   num_devices=NCORES)
    K = F_EMB + 1
    # lr[:, 0:R] = lhsT_aug ; lr[:, R:] = rhs_aug
    lr_ap = nc.dram_tensor("lr", [K, R + N], F32R, kind="ExternalInput").ap()
    bi_ap = nc.dram_tensor("bi", [128, IT], F32, kind="ExternalInput").ap()
    wo_ap = nc.dram_tensor("wo", [R, N], F32, kind="ExternalOutput").ap()

    with tile.TileContext(nc) as tc, ExitStack() as ctx:
        inp = ctx.enter_context(tc.tile_pool(name="inp", bufs=1))
        psp = ctx.enter_context(tc.tile_pool(name="psp", bufs=2, space="PSUM"))
        outp = ctx.enter_context(tc.tile_pool(name="outp", bufs=2))

        lr = inp.tile([K, R + N], F32R)
        nc.sync.dma_start(lr[:], lr_ap[:])
        bi = inp.tile([128, IT], F32)
        nc.sync.dma_start(bi[:], bi_ap[:])

        wo_r = wo_ap.rearrange("(i p) n -> p i n", p=128)   # [128, IT, N]
        for h in range(2):                                  # halves: 2 i-tiles
            wsb = outp.tile([128, 2 * N], F32, tag="wsb")
            for ii in range(2):
                i = 2 * h + ii
                ps = psp.tile([128, N], F32, tag="ps")      # 4 PSUM banks
                for j in range(JT):
                    nc.tensor.matmul(
                        ps[:, j * 512:(j + 1) * 512],
                        lr[:, i * 128:(i + 1) * 128],
                        lr[:, R + j * 512:R + (j + 1) * 512],
                        start=True, stop=True,
                    )
                nc.scalar.activation(
                    wsb[:, ii * N:(ii + 1) * N], ps[:],
                    mybir.ActivationFunctionType.Sigmoid,
                    bias=bi[:, i:i + 1])
            nc.sync.dma_start(
                wo_r[:, 2 * h:2 * h + 2, :],
                wsb[:].rearrange("p (i n) -> p i n", i=2),
            )

    nc.compile()
    return nc


def _get_nc(key, builder):
    nc = _NC_CACHE.get(key)
    if nc is None:
        nc = builder()
        _NC_CACHE[key] = nc
    return nc


def _edge_index() -> np.ndarray:
    idx = np.arange(B * N * N, dtype=np.int32)
    rows = idx // N
    cols = idx % N + N * (rows // N)
    return np.stack([rows, cols]).astype(np.int32)


def kernel(x, A, W_embed, b_embed, temperature, threshold):
    x = np.asarray(x, dtype=np.float32)
    A = np.asarray(A, dtype=np.float32)
    W_embed = np.asarray(W_embed, dtype=np.float32)
    b_embed = np.asarray(b_embed, dtype=np.float32)
    T = np.float32(np.asarray(temperature).reshape(()))
    thr = np.abs(np.float32(np.asarray(threshold).reshape(())))

    # ---- launch 1: x_emb ----
    nc1 = _get_nc("p1", _build_phase1)
    wb = np.empty((F_IN, F_EMB + 1), dtype=np.float32)
    wb[:, :F_EMB] = W_embed
    wb[:, F_EMB] = 0.0
    wb[:F_EMB, F_EMB] = b_embed
    in1 = []
    for c in range(NCORES):
        b, rb = divmod(c, CPB)
        pk = np.empty((N, C1), dtype=np.float32)
        pk[:, :F_IN] = x[b]
        pk[:, F_IN:] = A[b, rb * R:(rb + 1) * R, :].T
        in1.append({"pk": pk, "wb": wb})
    res1 = run_bass_kernel_spmd(nc1, in1, core_ids=CORE_IDS)

    x_emb = np.empty((B, N, F_EMB), dtype=np.float32)
    for c in range(NCORES):
        b, rb = divmod(c, CPB)
        x_emb[b, rb * R:(rb + 1) * R, :] = res1.results[c]["et"].T

    # ---- host: global scale + fold constants ----
    centroid = x_emb.mean(axis=1, keepdims=True, dtype=np.float32)
    scale = np.float32(0.9) / np.abs(x_emb - centroid).max()
    s2 = np.float32(T * scale * scale)          # T * scale^2
    sq0 = np.einsum("bne,bne->bn", x_emb, x_emb).astype(np.float32)  # [B,N]

    nc2 = _get_nc("p2", _build_phase2)
    in2 = []
    for c in range(NCORES):
        b, rb = divmod(c, CPB)
        eT = x_emb[b].T                          # [E, N]
        lr = np.empty((F_EMB + 1, R + N), dtype=np.float32)
        lr[:F_EMB, :R] = (2.0 * s2) * eT[:, rb * R:(rb + 1) * R]
        lr[F_EMB, :R] = 1.0
        lr[:F_EMB, R:] = eT
        lr[F_EMB, R:] = (-s2) * sq0[b]
        bi = (T * thr - s2 * sq0[b, rb * R:(rb + 1) * R])
        bi = np.ascontiguousarray(bi.reshape(IT, 128).T)   # [128, IT]
        in2.append({"lr": lr, "bi": bi})
    res2 = run_bass_kernel_spmd(nc2, in2, core_ids=CORE_IDS)

    adj = np.empty((B, N, N), dtype=np.float32)
    for c in range(NCORES):
        b, rb = divmod(c, CPB)
        adj[b, rb * R:(rb + 1) * R, :] = res2.results[c]["wo"]

    return x_emb, _edge_index(), adj.reshape(-1)
